# revision 1
# baseline (speedup 1.0000x reference)
"""Decoder block Bass/Tile kernel for TRN2, SPMD over 8 cores.

Sharding: core c = (batch b = c//4, j = c%4). Each core:
  - computes LN1 + K,V for ALL T_kv tokens of its batch (redundant x4, zero comm)
  - handles 512 queries: chunk A = rows [256j, 256j+256), chunk B = rows
    [256(7-j), 256(7-j)+256)  (causal load balance)
  - attention klen padded to a uniform size (1024 for A, 2048 for B) with
    host-provided -60000 masks so the program is identical on all cores
  - proj + residual + LN2 + MLP + residual for its 512 rows
Host gathers the 8 [512, 1024] shards into the full output.

Layouts: "fm" = [feature(partition), token(free)], "rm" = [token, feature].
LN in rm (bn_stats), matmul inputs fm via fp16 DMA-transpose. Matmuls fp16
with fp32 PSUM accumulation. Residual stream fp32.
"""

from contextlib import ExitStack
from dataclasses import dataclass

import numpy as np

import concourse.bass as bass
import concourse.tile as tile
from concourse import mybir
from concourse._compat import with_exitstack

F32 = mybir.dt.float32
F16 = mybir.dt.float16
MASK_NEG = -60000.0


@dataclass
class Cfg:
    D: int = 1024
    DFF: int = 4096
    H: int = 16  # heads
    DH: int = 64  # head dim
    T_kv: int = 2048
    T_q: int = 512  # 2 chunks of CH
    CH: int = 256
    klenA_pad: int = 1024
    klenB_pad: int = 2048
    mmdt: str = "float16"

    @property
    def HP(self):  # head pairs
        return self.H // 2

    @property
    def VA(self):  # augmented V width (dv + ones column per head)
        return self.H * (self.DH + 1)

    @property
    def NKTA(self):
        return self.klenA_pad // 128

    @property
    def NKTB(self):
        return self.klenB_pad // 128

    @property
    def NKT(self):
        return self.NKTA + self.NKTB


def _bcast_ap(ap, p=128):
    """[N] dram AP -> [p, N] with partition stride 0."""
    return bass.AP(tensor=ap.tensor, offset=ap.offset, ap=[[0, p]] + list(ap.ap))


@with_exitstack
def decoder_kernel(ctx: ExitStack, tc: tile.TileContext, cfg: Cfg, io: dict):
    nc = tc.nc
    MD = getattr(mybir.dt, cfg.mmdt)
    D, DFF, H, DH = cfg.D, cfg.DFF, cfg.H, cfg.DH
    HP, VA, CH = cfg.HP, cfg.VA, cfg.CH
    T_kv, T_q = cfg.T_kv, cfg.T_q
    ND = D // 128  # feature tiles
    NFF = DFF // 128
    NTKV = T_kv // 128
    NTQ = T_q // 128
    W2 = 2 * CH  # paired-head free width (512)

    # V chunk width for psum (<=512); VA = H*65
    n_vch = (VA + 511) // 512
    while VA % n_vch != 0:
        n_vch += 1
    VCH = VA // n_vch
    assert VCH <= 512

    const = ctx.enter_context(tc.tile_pool(name="const", bufs=1))
    eps_t = const.tile([128, 1], F32)
    nc.vector.memset(eps_t, 1e-5)
    ones_t = const.tile([1, 64], F32)
    nc.vector.memset(ones_t, 1.0)
    bq_sb = const.tile([128, ND], F32)
    nc.gpsimd.dma_start(out=bq_sb, in_=io["bq"].rearrange("(t p) -> p t", p=128))
    bk_sb = const.tile([128, ND], F32)
    nc.gpsimd.dma_start(out=bk_sb, in_=io["bk"].rearrange("(t p) -> p t", p=128))
    bfc1_sb = const.tile([128, NFF], F32)
    nc.gpsimd.dma_start(out=bfc1_sb, in_=io["bfc1"].rearrange("(t p) -> p t", p=128))
    vb_sb = const.tile([128, VA], F32)
    nc.gpsimd.dma_start(out=vb_sb, in_=_bcast_ap(io["vb"]))

    # ---------------- persistent activations ----------------
    acts = ctx.enter_context(tc.tile_pool(name="acts", bufs=1))
    K_sb = [acts.tile([128, T_kv], MD, tag=f"K{d}", name=f"K{d}") for d in range(ND)]
    Q_sb = [acts.tile([128, 2 * T_q], MD, tag=f"Q{d}", name=f"Q{d}") for d in range(ND)]
    for d in range(ND):
        nc.vector.memset(Q_sb[d], 0.0)
    V_sb = [acts.tile([128, VA], MD, tag=f"V{t}", name=f"V{t}") for t in range(NTKV)]
    O_sb = [acts.tile([128, T_q], MD, tag=f"O{h}", name=f"O{h}") for h in range(HP)]

    # ---------------- LN + transpose helper ----------------
    def ln_transpose(src_dram, src_sb, n_tiles, fm_tiles, pool, stats, tagp):
        for rt in range(n_tiles):
            if src_dram is not None:
                x_t = pool.tile([128, D], F32, tag=f"{tagp}_in")
                nc.gpsimd.dma_start(
                    out=x_t, in_=src_dram[rt * 128 : (rt + 1) * 128, :]
                )
            else:
                x_t = src_sb[rt]
            nsub = D // 512
            st = stats.tile([128, nsub, 6], F32, tag="ln_st")
            for s in range(nsub):
                nc.vector.bn_stats(
                    out=st[:, s, :], in_=x_t[:, s * 512 : (s + 1) * 512]
                )
            mv = stats.tile([128, 2], F32, tag="ln_mv")
            nc.vector.bn_aggr(out=mv, in_=st)
            sd = stats.tile([128, 1], F32, tag="ln_sd")
            nc.scalar.activation(
                out=sd, in_=mv[:, 1:2],
                func=mybir.ActivationFunctionType.Sqrt, bias=eps_t,
            )
            rec = stats.tile([128, 1], F32, tag="ln_rec")
            nc.vector.reciprocal(out=rec, in_=sd)
            xh = pool.tile([128, D], MD, tag=f"{tagp}_xh")
            nc.vector.tensor_scalar(
                out=xh, in0=x_t, scalar1=mv[:, 0:1], scalar2=rec,
                op0=mybir.AluOpType.subtract, op1=mybir.AluOpType.mult,
            )
            for d in range(ND):
                dst = fm_tiles[d][rt // 4][:, (rt % 4) * 128 : (rt % 4 + 1) * 128]
                if mybir.dt.size(MD) == 2:
                    eng = nc.sync if (rt * ND + d) % 2 == 0 else nc.scalar
                    eng.dma_start_transpose(
                        out=dst, in_=xh[:, d * 128 : (d + 1) * 128]
                    )
                else:  # fp32 debug path: strided-AP transpose via plain DMA
                    nc.sync.dma_start(
                        out=dst,
                        in_=xh[:, d * 128 : (d + 1) * 128].rearrange("a b -> b a"),
                    )

    with tc.tile_pool(name="fm", bufs=1) as fmp:
        NCHKV, NCHQ = T_kv // 512, T_q // 512
        xkv_fm = [
            [fmp.tile([128, 512], MD, tag=f"xkvfm{d}_{c}", name=f"xkvfm{d}_{c}")
             for c in range(NCHKV)]
            for d in range(ND)
        ]
        xq_fm = [
            [fmp.tile([128, 512], MD, tag=f"xqfm{d}_{c}", name=f"xqfm{d}_{c}")
             for c in range(NCHQ)]
            for d in range(ND)
        ]
        # ---------------- QKV projections ----------------
        with tc.tile_pool(name="wqk", bufs=3) as wqk, tc.tile_pool(
            name="wv", bufs=1
        ) as wvp, tc.tile_pool(name="psqkv", bufs=4, space="PSUM") as psq:
            # V weights load FIRST (head of the gpsimd DMA queue, no deps)
            wv_sb = [wvp.tile([128, VA], MD, tag=f"wv{kt}", name=f"wv{kt}") for kt in range(ND)]
            for kt in range(ND):
                nc.gpsimd.dma_start(
                    out=wv_sb[kt], in_=io["wv"][kt * 128 : (kt + 1) * 128, :]
                )
            with tc.tile_pool(name="ln1", bufs=3) as lnp, tc.tile_pool(
                name="ln1st", bufs=4
            ) as lnst:
                ln_transpose(io["x_kv"], None, NTKV, xkv_fm, lnp, lnst, "kv")
                ln_transpose(io["x_q"], None, NTQ, xq_fm, lnp, lnst, "q")

            # V (Option 1, consumes fm tiles in LN emission order)
            for tt in range(NTKV):
                for ch in range(n_vch):
                    ps = psq.tile([128, VCH], F32, tag="psv")
                    for kt in range(ND):
                        nc.tensor.matmul(
                            ps,
                            xkv_fm[kt][tt // 4][:, (tt % 4) * 128 : (tt % 4 + 1) * 128],
                            wv_sb[kt][:, ch * VCH : (ch + 1) * VCH],
                            start=(kt == 0),
                            stop=(kt == ND - 1),
                        )
                    nc.vector.tensor_add(
                        out=V_sb[tt][:, ch * VCH : (ch + 1) * VCH],
                        in0=ps,
                        in1=vb_sb[:, ch * VCH : (ch + 1) * VCH],
                    )
            # K then Q (Option 2: weights stationary, fm out)
            for which, wname, bias_sb, fm_src, out_sb, T in (
                ("k", "wk", bk_sb, xkv_fm, K_sb, T_kv),
                ("q", "wq", bq_sb, xq_fm, Q_sb, T_q),
            ):
                for do in range(ND):
                    wb = wqk.tile([128, ND, 128], MD, tag="wqk")
                    nc.gpsimd.dma_start(
                        out=wb,
                        in_=io[wname][:, do * 128 : (do + 1) * 128].rearrange(
                            "(kt p) c -> p kt c", p=128
                        ),
                    )
                    for tch in range(T // 512):
                        ps = psq.tile([128, 512], F32, tag="psqk")
                        for kt in range(ND):
                            nc.tensor.matmul(
                                ps,
                                wb[:, kt, :],
                                fm_src[kt][tch],
                                start=(kt == 0),
                                stop=(kt == ND - 1),
                            )
                        if which == "k":
                            nc.scalar.activation(
                                out=out_sb[do][:, tch * 512 : (tch + 1) * 512],
                                in_=ps,
                                func=mybir.ActivationFunctionType.Identity,
                                bias=bias_sb[:, do : do + 1],
                            )
                        else:
                            # Q: scatter into per-(chunk, head) blocks with the
                            # complementary head's partitions left zero
                            for ci in range(2):
                                for h in range(2):
                                    blk = (2 * ci + h) * CH
                                    nc.scalar.activation(
                                        out=out_sb[do][
                                            h * 64 : (h + 1) * 64,
                                            blk : blk + CH,
                                        ],
                                        in_=ps[
                                            h * 64 : (h + 1) * 64,
                                            ci * CH : (ci + 1) * CH,
                                        ],
                                        func=mybir.ActivationFunctionType.Identity,
                                        bias=bias_sb[h * 64 : (h + 1) * 64, do : do + 1],
                                    )

    # ---------------- attention + proj ----------------
    mid = ctx.enter_context(tc.tile_pool(name="mid", bufs=1))
    x2_sb = [mid.tile([128, D], F32, tag=f"x2_{t}", name=f"x2_{t}") for t in range(NTQ)]
    xq2_fm = [
        [mid.tile([128, 512], MD, tag=f"xq2fm{d}_{c}", name=f"xq2fm{d}_{c}")
         for c in range(T_q // 512)]
        for d in range(ND)
    ]
    rscr = nc.dram_tensor("rscratch", [2 * HP * 2, CH], F32).ap()
    chunks = [(0, cfg.NKTA, 0), (1, cfg.NKTB, cfg.NKTA)]  # (ci, nkt, mask_off)
    with tc.tile_pool(name="attn_w", bufs=1) as awp:
        # prefetch wproj while attention runs
        wproj_sb = [awp.tile([128, D], MD, tag=f"wp{d}", name=f"wp{d}") for d in range(ND)]
        for d in range(ND):
            nc.gpsimd.dma_start(
                out=wproj_sb[d], in_=io["wproj"][d * 128 : (d + 1) * 128, :]
            )
        with tc.tile_pool(name="attn_m", bufs=1) as mp, tc.tile_pool(
            name="attn_p", bufs=4
        ) as pp, tc.tile_pool(name="attn_ps", bufs=4, space="PSUM"
        ) as aps, tc.tile_pool(name="attn_po", bufs=4, space="PSUM"
        ) as ops:
            for ci, nkt, moff in chunks:
                cc = slice(ci * CH, (ci + 1) * CH)
                masks = []
                for k in range(nkt):
                    m = mp.tile([128, W2], MD, tag=f"mask{ci}_{k}")
                    nc.gpsimd.dma_start(out=m, in_=io["masks"][moff + k, :, :])
                    masks.append(m)
                for hp in range(HP):
                    po = [ops.tile([128, CH], F32, tag="po", name="po") for _ in range(2)]
                    for kti in range(nkt):
                        ps = aps.tile([128, W2], F32, tag="ps_s")
                        kcol = slice(kti * 128, (kti + 1) * 128)
                        for h in range(2):
                            blk = (2 * ci + h) * CH
                            nc.tensor.matmul(
                                ps[:, h * CH : (h + 1) * CH],
                                K_sb[hp][:, kcol],
                                Q_sb[hp][:, blk : blk + CH],
                                start=True, stop=True,
                            )
                        if not (ci == 1 and (kti + 1) * 128 <= cfg.klenB_pad // 2):
                            nc.vector.tensor_add(
                                out=ps, in0=ps, in1=masks[kti]
                            )
                        pt = pp.tile([128, W2], MD, tag="pt")
                        nc.scalar.activation(
                            out=pt, in_=ps,
                            func=mybir.ActivationFunctionType.Exp,
                        )
                        for h in range(2):
                            hg = 2 * hp + h
                            nc.tensor.matmul(
                                po[h][0:65, :],
                                V_sb[kti][:, hg * 65 : hg * 65 + 65],
                                pt[:, h * CH : (h + 1) * CH],
                                start=(kti == 0),
                                stop=(kti == nkt - 1),
                            )
                    # normalize + evict
                    for h in range(2):
                        slot = (ci * HP + hp) * 2 + h
                        r = pp.tile([1, CH], F32, tag="recip")
                        nc.vector.reciprocal(out=r, in_=po[h][64:65, :])
                        nc.sync.dma_start(
                            out=rscr[slot : slot + 1, :], in_=r
                        )
                        # evict numerator scaled by 1/4096 (fits fp16);
                        # normalization happens in one batch at attention end
                        nc.scalar.activation(
                            out=O_sb[hp][h * 64 : (h + 1) * 64, cc],
                            in_=po[h][0:64, :],
                            func=mybir.ActivationFunctionType.Copy,
                            scale=1.0 / 4096.0,
                        )
            # batch normalization of O: bc = 4096/den broadcast via DRAM
            for ci, _, _ in chunks:
                cc = slice(ci * CH, (ci + 1) * CH)
                for hp in range(HP):
                    bc_sb = pp.tile([128, CH], F32, tag="bcsb")
                    for h in range(2):
                        slot = (ci * HP + hp) * 2 + h
                        nc.sync.dma_start(
                            out=bc_sb[h * 64 : (h + 1) * 64, :],
                            in_=bass.AP(
                                tensor=rscr.tensor,
                                offset=rscr.offset + slot * CH,
                                ap=[[0, 64], [1, CH]],
                            ),
                        )
                    nc.vector.tensor_mul(
                        out=O_sb[hp][:, cc], in0=O_sb[hp][:, cc], in1=bc_sb
                    )

        # ---------------- proj + residual ----------------
        with tc.tile_pool(name="proj", bufs=3) as prp, tc.tile_pool(
            name="projps", bufs=4, space="PSUM"
        ) as prps:
            for qt in range(NTQ):
                x_t = prp.tile([128, D], F32, tag="xq_res")
                nc.gpsimd.dma_start(
                    out=x_t, in_=io["x_q"][qt * 128 : (qt + 1) * 128, :]
                )
                for ch2 in range(D // 512):
                    ps = prps.tile([128, 512], F32, tag="pspr")
                    for hp in range(ND):
                        nc.tensor.matmul(
                            ps,
                            O_sb[hp][:, qt * 128 : (qt + 1) * 128],
                            wproj_sb[hp][:, ch2 * 512 : (ch2 + 1) * 512],
                            start=(hp == 0),
                            stop=(hp == ND - 1),
                        )
                    nc.vector.tensor_add(
                        out=x2_sb[qt][:, ch2 * 512 : (ch2 + 1) * 512],
                        in0=ps,
                        in1=x_t[:, ch2 * 512 : (ch2 + 1) * 512],
                    )

    # ---------------- LN2 + transpose ----------------
    with tc.tile_pool(name="ln2", bufs=3) as ln2p, tc.tile_pool(
        name="ln2st", bufs=4
    ) as ln2st:
        ln_transpose(None, x2_sb, NTQ, xq2_fm, ln2p, ln2st, "l2")

    # ---------------- fc1 + gelu + fc2 (pipelined) ----------------
    ghp = ctx.enter_context(tc.tile_pool(name="gh", bufs=1))
    gh_sb = [ghp.tile([128, T_q], MD, tag=f"gh{f}", name=f"gh{f}") for f in range(NFF)]
    with tc.tile_pool(name="fc1w", bufs=3) as f1w, tc.tile_pool(
        name="fc2w", bufs=3
    ) as f2w, tc.tile_pool(name="fc2out", bufs=3) as f2o, tc.tile_pool(
        name="fcps", bufs=3, space="PSUM"
    ) as fps, tc.tile_pool(name="fc2acc", bufs=1, space="PSUM") as f2ps:
        wb2_tiles = {}
        for sweep in range(2):
            accs = {}
            for qt in range(NTQ):
                accs[qt] = f2ps.tile(
                    [128, 512], F32, tag=f"acc{qt}", name=f"acc{qt}"
                )
            for ff in range(NFF):
                if sweep == 0:
                    wb = f1w.tile([128, ND, 128], MD, tag="wfc1")
                    nc.gpsimd.dma_start(
                        out=wb,
                        in_=io["wfc1"][:, ff * 128 : (ff + 1) * 128].rearrange(
                            "(kt p) c -> p kt c", p=128
                        ),
                    )
                    ps = fps.tile([128, T_q], F32, tag="psf1")
                    for kt in range(ND):
                        nc.tensor.matmul(
                            ps, wb[:, kt, :], xq2_fm[kt][0],
                            start=(kt == 0), stop=(kt == ND - 1),
                        )
                    nc.scalar.activation(
                        out=gh_sb[ff], in_=ps,
                        func=mybir.ActivationFunctionType.Gelu,
                        bias=bfc1_sb[:, ff : ff + 1],
                    )
                wb2 = f2w.tile([128, 512], MD, tag="wfc2")
                nc.gpsimd.dma_start(
                    out=wb2,
                    in_=io["wfc2"][
                        ff * 128 : (ff + 1) * 128, sweep * 512 : (sweep + 1) * 512
                    ],
                )
                for qt in range(NTQ):
                    nc.tensor.matmul(
                        accs[qt],
                        gh_sb[ff][:, qt * 128 : (qt + 1) * 128],
                        wb2,
                        start=(ff == 0),
                        stop=(ff == NFF - 1),
                    )
            for qt in range(NTQ):
                o = f2o.tile([128, 512], F32, tag="osb")
                nc.vector.tensor_add(
                    out=o,
                    in0=accs[qt],
                    in1=x2_sb[qt][:, sweep * 512 : (sweep + 1) * 512],
                )
                nc.sync.dma_start(
                    out=io["out"][
                        qt * 128 : (qt + 1) * 128,
                        sweep * 512 : (sweep + 1) * 512,
                    ],
                    in_=o,
                )


def split_drain_waits(nc):
    """walrus CoreV3 rejects >1 sync wait on several instruction types;
    split extras into single-wait NOPs preceding the instruction on the
    same (in-order) engine."""
    idx = 0

    def fix_block(b):
        nonlocal idx
        new = []
        changed = False
        for inst in b.instructions:
            si = inst.sync_info
            if si is not None and si.on_wait and len(si.on_wait) > 1:
                waits = list(si.on_wait)
                for w in waits[:-1]:
                    idx += 1
                    nop = mybir.InstNoOp(
                        name=f"I-dsplit-{idx}",
                        sync_info=mybir.SyncInfo(on_wait=[w], on_update=[]),
                    )
                    nop.engine = inst.engine
                    new.append(nop)
                inst.sync_info = mybir.SyncInfo(
                    on_wait=[waits[-1]], on_update=list(si.on_update or [])
                )
                changed = True
            new.append(inst)
        if changed:
            b.instructions = new

    for f in nc.m.functions:
        for b in f.blocks:
            fix_block(b)


def declare_io(nc, cfg: Cfg):
    c = cfg
    WD = getattr(mybir.dt, c.mmdt)
    spec = {
        "x_kv": ([c.T_kv, c.D], F32, False),
        "x_q": ([c.T_q, c.D], F32, False),
        "wq": ([c.D, c.D], WD, False),
        "wk": ([c.D, c.D], WD, False),
        "wv": ([c.D, c.VA], WD, False),
        "bq": ([c.D], F32, False),
        "bk": ([c.D], F32, False),
        "vb": ([c.VA], F32, False),
        "wproj": ([c.D, c.D], WD, False),
        "wfc1": ([c.D, c.DFF], WD, False),
        "bfc1": ([c.DFF], F32, False),
        "wfc2": ([c.DFF, c.D], WD, False),
        "masks": ([c.NKT, 128, 2 * c.CH], WD, False),
        "out": ([c.T_q, c.D], F32, True),
    }
    io = {}
    for name, (shape, dt, is_out) in spec.items():
        io[name] = nc.declare_dram_parameter(name, shape, dt, isOutput=is_out).ap()
    return io


def build(cfg: Cfg, split: bool = True):
    nc = bass.Bass(num_devices=8)
    io = declare_io(nc, cfg)
    with tile.TileContext(nc) as tc:
        decoder_kernel(tc, cfg, io)
    if split:
        split_drain_waits(nc)
    return nc


# ======================= host-side prep =======================


def make_masks(cfg: Cfg, qgA, qgB):
    """[NKT, 128, 2*CH] fp16-ish: 0 where key k <= query q (valid), else
    -60000. Duplicated for the 2 heads along the free dim."""
    m = np.full((cfg.NKT, 128, 2 * cfg.CH), MASK_NEG, np.float32)
    for ci, (qg, nkt, off) in enumerate(
        [(qgA, cfg.NKTA, 0), (qgB, cfg.NKTB, cfg.NKTA)]
    ):
        q = qg + np.arange(cfg.CH)[None, :]
        for k in range(nkt):
            kg = k * 128 + np.arange(128)[:, None]
            valid = (kg <= q).astype(np.float32)
            blk = (1.0 - valid) * MASK_NEG
            m[off + k, :, 0 : cfg.CH] = blk
            m[off + k, :, cfg.CH : 2 * cfg.CH] = blk
    return m.astype(np.float16)


def host_prep(cfg: Cfg, x, ln1_g, ln1_b, w_qkv, w_proj, ln2_g, ln2_b, w_fc1, w_fc2):
    """Returns (in_maps list of 8 dicts, assemble(results)->full out)."""
    D, H, DH = cfg.D, cfg.H, cfg.DH
    x = np.asarray(x, np.float32)
    B = x.shape[0]
    w_qkv = np.asarray(w_qkv, np.float32)
    bqkv = np.asarray(ln1_b, np.float32) @ w_qkv  # [3D]
    w_qkv = w_qkv * np.asarray(ln1_g, np.float32)[:, None]
    bq = bqkv[0:D] / np.sqrt(DH).astype(np.float32)
    bk = bqkv[D : 2 * D]
    bv = bqkv[2 * D : 3 * D]
    wq = w_qkv[:, 0:D] / np.sqrt(DH).astype(np.float32)
    wk = w_qkv[:, D : 2 * D]
    wv = w_qkv[:, 2 * D : 3 * D]
    wv_aug = np.zeros((D, cfg.VA), np.float32)
    vb_aug = np.zeros((cfg.VA,), np.float32)
    for h in range(H):
        wv_aug[:, h * (DH + 1) : h * (DH + 1) + DH] = wv[:, h * DH : (h + 1) * DH]
        vb_aug[h * (DH + 1) : h * (DH + 1) + DH] = bv[h * DH : (h + 1) * DH]
        vb_aug[h * (DH + 1) + DH] = 1.0 / 4096.0
    bfc1 = np.asarray(ln2_b, np.float32) @ np.asarray(w_fc1, np.float32)
    wfc1 = np.asarray(w_fc1, np.float32) * np.asarray(ln2_g, np.float32)[:, None]

    wd = np.float32 if cfg.mmdt == "float32" else np.float16
    weights = {
        "wq": wq.astype(wd),
        "wk": wk.astype(wd),
        "wv": wv_aug.astype(wd),
        "bq": bq.astype(np.float32),
        "bk": bk.astype(np.float32),
        "vb": vb_aug.astype(np.float32),
        "wproj": np.asarray(w_proj, np.float32).astype(wd),
        "wfc1": wfc1.astype(wd),
        "bfc1": bfc1.astype(np.float32),
        "wfc2": np.asarray(w_fc2, np.float32).astype(wd),
    }

    in_maps = []
    core_rows = []
    n_j = 4  # chunk pairs per batch
    for c in range(8):
        b, j = c // n_j, c % n_j
        qgA, qgB = cfg.CH * j, cfg.CH * (2 * n_j - 1 - j)
        rows = np.r_[qgA : qgA + cfg.CH, qgB : qgB + cfg.CH]
        core_rows.append((b, rows))
        im = dict(weights)
        im["x_kv"] = np.ascontiguousarray(x[b])
        im["x_q"] = np.ascontiguousarray(x[b][rows])
        im["masks"] = make_masks(cfg, qgA, qgB).astype(wd)
        in_maps.append(im)

    def assemble(results):
        out = np.zeros((B, x.shape[1], D), np.float32)
        for c, (b, rows) in enumerate(core_rows):
            out[b][rows] = results[c]["out"]
        return out

    return in_maps, assemble


# ======================= public entry point =======================

LAST_RESULTS = {}
_CACHE = {}


def kernel(x, ln1_g, ln1_b, w_qkv, w_proj, ln2_g, ln2_b, w_fc1, w_fc2,
           _trace=False):
    """Full-input decoder block on 8 TRN2 NeuronCores; returns full output."""
    from concourse.bass_utils import run_bass_kernel_spmd

    cfg = Cfg()
    in_maps, assemble = host_prep(
        cfg, x, ln1_g, ln1_b, w_qkv, w_proj, ln2_g, ln2_b, w_fc1, w_fc2
    )
    if "nc" not in _CACHE:
        _CACHE["nc"] = build(cfg)
    res = run_bass_kernel_spmd(
        _CACHE["nc"], in_maps, core_ids=list(range(8)), trace=_trace
    )
    LAST_RESULTS["res"] = res
    return assemble(res.results)



# revision 13
# speedup vs baseline: 1.2129x; 1.2129x over previous
"""Decoder block Bass/Tile kernel for TRN2, SPMD over 8 cores.

Sharding: core c = (batch b = c//4, j = c%4). Each core:
  - computes LN1 + K,V for ALL T_kv tokens of its batch (redundant x4, zero comm)
  - handles 512 queries: chunk A = rows [256j, 256j+256), chunk B = rows
    [256(7-j), 256(7-j)+256)  (causal load balance)
  - attention klen padded uniformly (1024 for A, 2048 for B) with
    host-provided masks so the program is identical on all cores
  - proj + residual + LN2 + MLP + residual for its 512 rows

Everything runs in "fm" layout ([feature(partition), token(free)]); the host
pre-transposes x (free) so the device never transposes. LayerNorm statistics
are computed on the PE (ones-column matmuls for sum / sum-of-squares),
rsqrt via ACT Ln+Exp (one table set shared with attention's Exp), and the
per-token (mean, rstd) are broadcast across partitions with outer-product
matmuls, then applied in place by DVE. Scores batch 3 k-tiles into one
3-bank PSUM mega-tile so each ACT Exp amortizes its 352-cycle fixed
overhead; attention is software-pipelined one group deep. Softmax
denominators ride as constant-1 columns of V (memset once) and are divided
out at eviction via a reciprocal broadcast into the unused upper partitions
of the same PSUM bank. Matmuls fp16 with fp32 PSUM accumulation; residual
stream fp32.
"""

from contextlib import ExitStack
from dataclasses import dataclass

import numpy as np

import concourse.bass as bass
import concourse.tile as tile
from concourse import mybir
from concourse._compat import with_exitstack

F32 = mybir.dt.float32
F16 = mybir.dt.float16
MASK_NEG = -60000.0
AF = mybir.ActivationFunctionType


@dataclass
class Cfg:
    D: int = 1024
    DFF: int = 4096
    H: int = 16
    DH: int = 64
    T_kv: int = 2048
    T_q: int = 512
    CH: int = 256
    klenA_pad: int = 1024
    klenB_pad: int = 2048
    mmdt: str = "float16"

    @property
    def HP(self):
        return self.H // 2

    @property
    def VA(self):  # per-head [64 dv | 1 den] interleaved
        return self.H * (self.DH + 1)

    @property
    def NKTA(self):
        return self.klenA_pad // 128

    @property
    def NKTB(self):
        return self.klenB_pad // 128


def _bcast_ap(ap, p=128):
    """[N] dram AP -> [p, N] with partition stride 0."""
    return bass.AP(tensor=ap.tensor, offset=ap.offset, ap=[[0, p]] + list(ap.ap))


def _groups(nkt, w=3):
    return [(g0, min(g0 + w, nkt)) for g0 in range(0, nkt, w)]


@with_exitstack
def decoder_kernel(ctx: ExitStack, tc: tile.TileContext, cfg: Cfg, io: dict):
    nc = tc.nc
    MD = getattr(mybir.dt, cfg.mmdt)
    D, DFF = cfg.D, cfg.DFF
    HP, VA, CH = cfg.HP, cfg.VA, cfg.CH
    T_kv, T_q = cfg.T_kv, cfg.T_q
    ND = D // 128
    NFF = DFF // 128
    NTKV = T_kv // 128
    NCH = T_kv // 512

    # ---------------- constants ----------------
    const = ctx.enter_context(tc.tile_pool(name="const", bufs=1))
    eps_t = const.tile([1, 1], F32)
    nc.vector.memset(eps_t, 1e-5)
    ones_col = const.tile([128, 1], MD)
    nc.vector.memset(ones_col, 1.0)
    ones_row = const.tile([1, 128], MD)
    nc.vector.memset(ones_row, 1.0)
    neg_row = const.tile([1, 128], MD)
    nc.vector.memset(neg_row, -1.0)
    bq_sb = const.tile([128, ND], F32)
    nc.sync.dma_start(out=bq_sb, in_=io["bq"].rearrange("(t p) -> p t", p=128))
    bk_sb = const.tile([128, ND], F32)
    nc.sync.dma_start(out=bk_sb, in_=io["bk"].rearrange("(t p) -> p t", p=128))
    bfc1_sb = const.tile([128, NFF], F32)
    nc.sync.dma_start(out=bfc1_sb, in_=io["bfc1"].rearrange("(t p) -> p t", p=128))
    vb_sb = const.tile([128, D], F32)
    nc.sync.dma_start(out=vb_sb, in_=_bcast_ap(io["vb"]))

    with tc.tile_pool(name="kqv_acts", bufs=1) as acts:
        K_sb = [acts.tile([128, T_kv], MD, tag=f"K{d}", name=f"K{d}")
                for d in range(ND)]
        Q_sb = [acts.tile([128, 2 * T_q], MD, tag=f"Q{d}", name=f"Q{d}")
                for d in range(ND)]
        for d in range(ND):
            nc.vector.memset(Q_sb[d], 0.0)
        V_sb = [acts.tile([128, VA], MD, tag=f"V{t}", name=f"V{t}")
                for t in range(NTKV)]
        for t in range(NTKV):  # constant denominator columns (one per head)
            nc.vector.memset(
                V_sb[t].rearrange("p (b c) -> p b c", c=65)[:, :, 64:65], 1.0
            )
        XQ32 = [acts.tile([128, T_q], F32, tag=f"XQ32_{d}", name=f"XQ32_{d}")
                for d in range(ND)]
        for d in range(ND):
            nc.gpsimd.dma_start(
                out=XQ32[d], in_=io["xq32"][d * 128 : (d + 1) * 128, :]
            )

        # =================== phase A+B: LN1 + QKV ===================
        with tc.tile_pool(name="xr", bufs=2) as xrp, tc.tile_pool(
            name="xsq", bufs=1
        ) as xsqp, tc.tile_pool(name="wv", bufs=1) as wvp, tc.tile_pool(
            name="wqk", bufs=4
        ) as wqkp, tc.tile_pool(name="xqc", bufs=1) as xqcp, tc.tile_pool(
            name="lnsm", bufs=1
        ) as lnsm, tc.tile_pool(name="stps", bufs=1, space="PSUM") as stps, \
            tc.tile_pool(name="bcps", bufs=1, space="PSUM") as bcps, \
            tc.tile_pool(name="qkvps", bufs=1, space="PSUM") as qkvps:
            wv_sb = [wvp.tile([128, D], MD, tag=f"wv{d}", name=f"wv{d}")
                     for d in range(ND)]
            for d in range(ND):
                nc.sync.dma_start(
                    out=wv_sb[d], in_=io["wv"][d * 128 : (d + 1) * 128, :]
                )

            def ln_stats(xtiles, sqtiles, tag):
                """Per-token LN stats from fm tiles. Returns (a16, nb16):
                [1,512] fp16 rstd and mean*rstd."""
                ps_s = stps.tile([1, 512], F32, tag="ps_s", name=f"ps_s{tag}")
                ps_q = stps.tile([1, 512], F32, tag="ps_q", name=f"ps_q{tag}")
                for d in range(ND):
                    nc.tensor.matmul(ps_s, ones_col, xtiles[d],
                                     start=(d == 0), stop=(d == ND - 1))
                for d in range(ND):
                    nc.tensor.matmul(ps_q, ones_col, sqtiles[d],
                                     start=(d == 0), stop=(d == ND - 1))
                mu = lnsm.tile([1, 512], F32, tag="mu", name=f"mu{tag}")
                nc.vector.tensor_scalar_mul(out=mu, in0=ps_s, scalar1=1.0 / D)
                msq = lnsm.tile([1, 512], F32, tag="msq", name=f"msq{tag}")
                nc.vector.tensor_scalar_mul(out=msq, in0=ps_q, scalar1=1.0 / D)
                # scratch in the (now free) stats psum banks
                nc.vector.tensor_mul(out=ps_s, in0=mu, in1=mu)
                nc.vector.tensor_sub(out=msq, in0=msq, in1=ps_s)
                nc.scalar.activation(out=ps_q, in_=msq, func=AF.Ln, bias=eps_t)
                a16 = lnsm.tile([1, 512], MD, tag="a16", name=f"a16{tag}",
                                bufs=2)
                nc.scalar.activation(out=a16, in_=ps_q, func=AF.Exp, scale=-0.5)
                nb16 = lnsm.tile([1, 512], MD, tag="nb16", name=f"nb16{tag}",
                                 bufs=2)
                nc.vector.tensor_mul(out=nb16, in0=mu, in1=a16)
                return a16, nb16

            def ln_bcast_apply(a16, nb16, xtiles, tag):
                a_bc = bcps.tile([128, 512], F32, tag="abc", name=f"abc{tag}")
                nc.tensor.matmul(a_bc, ones_row, a16, start=True, stop=True)
                b_bc = bcps.tile([128, 512], F32, tag="bbc", name=f"bbc{tag}")
                nc.tensor.matmul(b_bc, neg_row, nb16, start=True, stop=True)
                for d in range(ND):
                    nc.vector.tensor_mul(out=xtiles[d], in0=xtiles[d], in1=a_bc)
                    nc.vector.tensor_add(out=xtiles[d], in0=xtiles[d], in1=b_bc)

            # ---- Q-chunk LN (on a cast of xq32; applied in place on XQc) ----
            XQc = [xqcp.tile([128, T_q], MD, tag=f"xqc{d}", name=f"XQc{d}")
                   for d in range(ND)]
            xqsq = [xsqp.tile([128, 512], MD, tag=f"xsq{d}", name=f"xqsq{d}")
                    for d in range(ND)]
            for d in range(ND):
                nc.vector.tensor_copy(out=XQc[d], in_=XQ32[d])
                nc.scalar.activation(out=xqsq[d], in_=XQ32[d], func=AF.Square)
            aq, nbq = ln_stats(XQc, xqsq, "q")

            def load_chunk(c):
                xr = [xrp.tile([128, 512], MD, tag=f"xr{d}", name=f"xr{c}_{d}")
                      for d in range(ND)]
                for d in range(ND):
                    nc.gpsimd.dma_start(
                        out=xr[d],
                        in_=io["x_fm"][d * 128 : (d + 1) * 128,
                                       c * 512 : (c + 1) * 512],
                    )
                sq = [xsqp.tile([128, 512], MD, tag=f"xsq{d}", name=f"sq{c}_{d}")
                      for d in range(ND)]
                for d in range(ND):
                    nc.scalar.activation(out=sq[d], in_=xr[d], func=AF.Square)
                return xr, sq

            xr0, sq0 = load_chunk(0)
            a0, nb0 = ln_stats(xr0, sq0, "kv0")
            ln_bcast_apply(aq, nbq, XQc, "q")
            ln_bcast_apply(a0, nb0, xr0, "kv0")
            nc.sync.dma_start(out=io["dbg_x"], in_=xr0[0])

            def v_proj(c, xp):
                for kt in range(4 * c, 4 * c + 4):
                    for ch in range(2):
                        ps = qkvps.tile([128, 512], F32, tag=f"vps{ch}",
                                        name=f"vps{kt}_{ch}")
                        for d in range(ND):
                            nc.tensor.matmul(
                                ps,
                                xp[d][:, (kt % 4) * 128 : (kt % 4 + 1) * 128],
                                wv_sb[d][:, ch * 512 : (ch + 1) * 512],
                                start=(d == 0),
                                stop=(d == ND - 1),
                            )
                        # scatter 8 head-blocks of 64, skipping den columns
                        dst = V_sb[kt][:, ch * 520 : ch * 520 + 520].rearrange(
                            "p (b c) -> p b c", c=65
                        )[:, :, 0:64]
                        nc.vector.tensor_add(
                            out=dst,
                            in0=ps.rearrange("p (b c) -> p b c", c=64),
                            in1=vb_sb[:, ch * 512 : (ch + 1) * 512].rearrange(
                                "p (b c) -> p b c", c=64
                            ),
                        )

            def k_proj(c, xp):
                for do in range(ND):
                    wk_t = wqkp.tile([128, ND, 128], MD, tag="wqk",
                                     name=f"wk{c}_{do}")
                    nc.sync.dma_start(out=wk_t, in_=io["wk"][do])
                    ps = qkvps.tile([128, 512], F32, tag="kps", bufs=2,
                                    name=f"kps{c}_{do}")
                    for d in range(ND):
                        nc.tensor.matmul(
                            ps, wk_t[:, d, :], xp[d],
                            start=(d == 0), stop=(d == ND - 1),
                        )
                    nc.scalar.activation(
                        out=K_sb[do][:, c * 512 : (c + 1) * 512], in_=ps,
                        func=AF.Identity, bias=bk_sb[:, do : do + 1],
                    )

            xp_c = xr0
            for c in range(NCH):
                nxt = None
                if c + 1 < NCH:
                    xr1, sq1 = load_chunk(c + 1)
                    a1, nb1 = ln_stats(xr1, sq1, f"kv{c+1}")
                    nxt = (xr1, a1, nb1)
                v_proj(c, xp_c)
                if nxt is not None:
                    xr1, a1, nb1 = nxt
                    ln_bcast_apply(a1, nb1, xr1, f"kv{c+1}")
                k_proj(c, xp_c)
                if nxt is not None:
                    xp_c = xr1

            # ---- Q projection + scatter (with complementary-head zeros) ----
            for do in range(ND):
                wq_t = wqkp.tile([128, ND, 128], MD, tag="wqk", name=f"wq{do}")
                nc.sync.dma_start(out=wq_t, in_=io["wq"][do])
                ps = qkvps.tile([128, 512], F32, tag="kps", bufs=2,
                                name=f"qps{do}")
                for d in range(ND):
                    nc.tensor.matmul(
                        ps, wq_t[:, d, :], XQc[d],
                        start=(d == 0), stop=(d == ND - 1),
                    )
                for ci in range(2):
                    for h in range(2):
                        blk = (2 * ci + h) * CH
                        nc.scalar.activation(
                            out=Q_sb[do][h * 64 : (h + 1) * 64, blk : blk + CH],
                            in_=ps[h * 64 : (h + 1) * 64, ci * CH : (ci + 1) * CH],
                            func=AF.Identity,
                            bias=bq_sb[h * 64 : (h + 1) * 64, do : do + 1],
                        )

        # ============ right-side pools: MLP weight streams + fp32 state ======
        f1w = ctx.enter_context(tc.tile_pool(name="mlpw1", bufs=8, side="right"))
        f2w = ctx.enter_context(tc.tile_pool(name="mlpw2", bufs=12, side="right"))
        x2P = ctx.enter_context(tc.tile_pool(name="x2P", bufs=1, side="right"))
        x2cP = ctx.enter_context(tc.tile_pool(name="x2cP", bufs=1, side="right"))
        w1t = []
        for ff in range(NFF):
            t = f1w.tile([128, ND, 128], MD, tag="wfc1", name=f"wfc1_{ff}")
            nc.gpsimd.dma_start(out=t, in_=io["wfc1"][ff])
            w1t.append(t)
        w2t = {}
        for s in range(2):
            for ff in range(NFF):
                t = f2w.tile([128, 512], MD, tag="wfc2", name=f"wfc2_{s}_{ff}")
                nc.sync.dma_start(out=t, in_=io["wfc2"][s, ff])
                w2t[(s, ff)] = t
        x2_sb = [x2P.tile([128, T_q], F32, tag=f"x2_{d}", name=f"x2_{d}")
                 for d in range(ND)]
        X2c = [x2cP.tile([128, 512], MD, tag=f"x2c{d}", name=f"X2c{d}")
               for d in range(ND)]

        with tc.tile_pool(name="attw", bufs=1) as awp:
            wproj_sb = [awp.tile([128, D], MD, tag=f"wp{d}", name=f"wp{d}")
                        for d in range(ND)]
            for d in range(ND):
                nc.sync.dma_start(
                    out=wproj_sb[d], in_=io["wproj"][d * 128 : (d + 1) * 128, :]
                )
            O_sb = [awp.tile([128, T_q], MD, tag=f"O{h}", name=f"O{h}")
                    for h in range(HP)]

            # =================== phase C: attention ===================
            with tc.tile_pool(name="attm", bufs=1) as mp, tc.tile_pool(
                name="attpt", bufs=2
            ) as ptp, tc.tile_pool(name="attsm", bufs=2) as smp, tc.tile_pool(
                name="scps", bufs=2, space="PSUM"
            ) as scps, tc.tile_pool(name="pops", bufs=2, space="PSUM") as pops:
                maskA = mp.tile([128, cfg.NKTA * 256], MD, tag="maskA")
                nc.gpsimd.dma_start(out=maskA, in_=io["maskA"])
                maskB = mp.tile([128, cfg.NKTB * 256], MD, tag="maskB")
                nc.gpsimd.dma_start(out=maskB, in_=io["maskB"])
                masks = {0: maskA, 1: maskB}
                nc.sync.dma_start(out=io["dbg_mask"], in_=maskA)



                for ci, nkt in ((0, cfg.NKTA), (1, cfg.NKTB)):
                    mask_lo = 0 if ci == 0 else 8
                    grs = _groups(nkt, 3)
                    for hp in range(HP):
                        po = pops.tile([128, 512], F32, tag="po",
                                       name=f"po{ci}_{hp}")
                        pend = []

                        def emit_S(gi, po=po, grs=grs, ci=ci, hp=hp,
                                   mask_lo=mask_lo, pend=pend):
                            g0, g1 = grs[gi]
                            w = (g1 - g0) * 512
                            sc = scps.tile([128, 1536], F32, tag="sc",
                                           name=f"sc{ci}_{hp}_{gi}")
                            for k in range(g0, g1):
                                nc.tensor.matmul(
                                    sc[:, (k - g0) * 512 : (k - g0 + 1) * 512],
                                    K_sb[hp][:, k * 128 : (k + 1) * 128],
                                    Q_sb[hp][:, ci * 512 : (ci + 1) * 512],
                                    start=True, stop=True,
                                )
                            r0, r1 = max(g0, mask_lo), g1
                            if r0 < r1:
                                mk = masks[ci][:, r0 * 256 : r1 * 256
                                               ].rearrange("p (t q) -> p t q",
                                                           q=256)
                                for h in range(2):
                                    scv = sc[:, (r0 - g0) * 512 : (r1 - g0) * 512
                                             ].rearrange("p (t h q) -> p t h q",
                                                         h=2, q=256)[:, :, h, :]
                                    nc.vector.tensor_add(
                                        out=scv, in0=scv, in1=mk,
                                    )
                            pt = ptp.tile([128, 1536], MD, tag="pt",
                                          name=f"pt{ci}_{hp}_{gi}")
                            nc.scalar.activation(
                                out=pt[:, 0:w], in_=sc[:, 0:w], func=AF.Exp
                            )
                            if ci == 0 and hp == 0 and gi == 0:
                                nc.sync.dma_start(out=io["dbg_pt"], in_=pt)
                            pend.append((pt, g0, g1))

                        def emit_AV(po=po, ci=ci, hp=hp, nkt=nkt, pend=pend):
                            pt, g0, g1 = pend.pop(0)
                            for k in range(g0, g1):
                                for h in range(2):
                                    hg = 2 * hp + h
                                    # h0+h1 are ONE accumulation group: a
                                    # start=True resets the whole bank's
                                    # has_written, so per-head groups would
                                    # erase each other's k=0 contribution.
                                    nc.tensor.matmul(
                                        po[0:65, h * CH : (h + 1) * CH],
                                        V_sb[k][:, hg * 65 : hg * 65 + 65],
                                        pt[:, (k - g0) * 512 + h * CH :
                                           (k - g0) * 512 + (h + 1) * CH],
                                        start=(k == 0 and h == 0),
                                        stop=(k == nkt - 1 and h == 1),
                                        skip_group_check=True,
                                    )

                        emit_S(0)
                        for gi in range(1, len(grs)):
                            emit_S(gi)
                            emit_AV()
                        emit_AV()

                        # drain: divide by the denominator accumulated in row 64
                        den = smp.tile([1, 512], MD, tag="den",
                                       name=f"den{ci}_{hp}")
                        nc.vector.tensor_copy(out=den, in_=po[64:65, :])
                        rec = smp.tile([1, 512], MD, tag="rec",
                                       name=f"rec{ci}_{hp}")
                        with nc.allow_low_precision(reason="softmax denom fp16"):
                            nc.vector.reciprocal(out=rec, in_=den)
                        for h in range(2):
                            nc.tensor.matmul(
                                po[64:128, h * CH : (h + 1) * CH],
                                ones_row[0:1, 0:64],
                                rec[0:1, h * CH : (h + 1) * CH],
                                start=True, stop=True,
                            )
                        slot = ci * HP + hp
                        nc.sync.dma_start(out=io["dbg_den"][slot, 0], in_=den)
                        nc.sync.dma_start(out=io["dbg_den"][slot, 1], in_=rec)
                        rb = smp.tile([64, 512], MD, tag="rb",
                                      name=f"rb{ci}_{hp}")
                        nc.vector.tensor_copy(out=rb, in_=po[64:128, :])
                        for h in range(2):
                            nc.vector.tensor_mul(
                                out=O_sb[hp][h * 64 : (h + 1) * 64,
                                             ci * CH : (ci + 1) * CH],
                                in0=po[0:64, h * CH : (h + 1) * CH],
                                in1=rb[:, h * CH : (h + 1) * CH],
                            )

            nc.sync.dma_start(out=io["dbg_k"], in_=K_sb[0])
            nc.sync.dma_start(out=io["dbg_q"], in_=Q_sb[0])
            nc.sync.dma_start(out=io["dbg_v"], in_=V_sb[0])
            nc.sync.dma_start(out=io["dbg_o"], in_=O_sb[0])

            # ============ phase D: proj + residual + LN2 ============
            with tc.tile_pool(name="prps", bufs=2, space="PSUM") as prps, \
                tc.tile_pool(name="ln2sm", bufs=1) as ln2sm, tc.tile_pool(
                    name="st2ps", bufs=1, space="PSUM") as st2ps, tc.tile_pool(
                    name="bc2ps", bufs=1, space="PSUM") as bc2ps:
                x2sq = [ln2sm.tile([128, 512], MD, tag=f"x2sq{d}",
                                   name=f"x2sq{d}") for d in range(ND)]
                for do in range(ND):
                    pp = prps.tile([128, 512], F32, tag="pp", name=f"pp{do}")
                    for hp in range(HP):
                        nc.tensor.matmul(
                            pp, wproj_sb[hp][:, do * 128 : (do + 1) * 128],
                            O_sb[hp][:, 0:T_q],
                            start=(hp == 0), stop=(hp == HP - 1),
                        )
                    nc.vector.tensor_add(out=x2_sb[do], in0=pp, in1=XQ32[do])
                    nc.vector.tensor_copy(out=X2c[do], in_=x2_sb[do])
                    nc.scalar.activation(out=x2sq[do], in_=x2_sb[do],
                                         func=AF.Square)

                ps_s = st2ps.tile([1, 512], F32, tag="ps_s2")
                ps_q = st2ps.tile([1, 512], F32, tag="ps_q2")
                for d in range(ND):
                    nc.tensor.matmul(ps_s, ones_col, X2c[d],
                                     start=(d == 0), stop=(d == ND - 1))
                for d in range(ND):
                    nc.tensor.matmul(ps_q, ones_col, x2sq[d],
                                     start=(d == 0), stop=(d == ND - 1))
                mu = ln2sm.tile([1, 512], F32, tag="mu2")
                nc.vector.tensor_scalar_mul(out=mu, in0=ps_s, scalar1=1.0 / D)
                msq = ln2sm.tile([1, 512], F32, tag="msq2")
                nc.vector.tensor_scalar_mul(out=msq, in0=ps_q, scalar1=1.0 / D)
                nc.vector.tensor_mul(out=ps_s, in0=mu, in1=mu)
                nc.vector.tensor_sub(out=msq, in0=msq, in1=ps_s)
                nc.scalar.activation(out=ps_q, in_=msq, func=AF.Ln, bias=eps_t)
                a16 = ln2sm.tile([1, 512], MD, tag="a162")
                nc.scalar.activation(out=a16, in_=ps_q, func=AF.Exp, scale=-0.5)
                nb16 = ln2sm.tile([1, 512], MD, tag="nb162")
                nc.vector.tensor_mul(out=nb16, in0=mu, in1=a16)
                a_bc = bc2ps.tile([128, 512], F32, tag="abc2")
                nc.tensor.matmul(a_bc, ones_row, a16, start=True, stop=True)
                b_bc = bc2ps.tile([128, 512], F32, tag="bbc2")
                nc.tensor.matmul(b_bc, neg_row, nb16, start=True, stop=True)
                for d in range(ND):
                    nc.vector.tensor_mul(out=X2c[d], in0=X2c[d], in1=a_bc)
                    nc.vector.tensor_add(out=X2c[d], in0=X2c[d], in1=b_bc)

    # =================== phase E: MLP ===================
    with tc.tile_pool(name="gh", bufs=1) as ghp, tc.tile_pool(
        name="ostg", bufs=2
    ) as ostg, tc.tile_pool(name="f1ps", bufs=2, space="PSUM") as fps, \
            tc.tile_pool(name="accps", bufs=1, space="PSUM") as aps:
        gh_sb = [ghp.tile([128, T_q], MD, tag=f"gh{f}", name=f"gh{f}")
                 for f in range(NFF)]
        acc = [aps.tile([128, 512], F32, tag=f"acc{dt}", name=f"acc{dt}")
               for dt in range(4)]

        def fc1(ff):
            ps1 = fps.tile([128, T_q], F32, tag="ps1", name=f"ps1_{ff}")
            for d in range(ND):
                nc.tensor.matmul(ps1, w1t[ff][:, d, :], X2c[d],
                                 start=(d == 0), stop=(d == ND - 1))
            nc.scalar.activation(out=gh_sb[ff], in_=ps1, func=AF.Gelu,
                                 bias=bfc1_sb[:, ff : ff + 1])

        def fc2(s, ff, accs):
            for dt in range(4):
                nc.tensor.matmul(
                    accs[dt],
                    w2t[(s, ff)][:, dt * 128 : (dt + 1) * 128],
                    gh_sb[ff],
                    start=(ff == 0), stop=(ff == NFF - 1),
                )

        fc1(0)
        for ff in range(1, NFF):
            fc1(ff)
            fc2(0, ff - 1, acc)
        fc2(0, NFF - 1, acc)
        for dt in range(4):
            o = ostg.tile([128, 512], F32, tag="ostg", name=f"o{dt}")
            nc.vector.tensor_add(out=o, in0=acc[dt], in1=x2_sb[dt])
            nc.gpsimd.dma_start(
                out=io["out"][dt * 128 : (dt + 1) * 128, :], in_=o
            )
        acc2 = [aps.tile([128, 512], F32, tag=f"acc{dt}", name=f"acc2_{dt}")
                for dt in range(4)]
        for ff in range(NFF):
            fc2(1, ff, acc2)
        for dt in range(4):
            o = ostg.tile([128, 512], F32, tag="ostg", name=f"o2_{dt}")
            nc.vector.tensor_add(out=o, in0=acc2[dt], in1=x2_sb[4 + dt])
            nc.gpsimd.dma_start(
                out=io["out"][(4 + dt) * 128 : (4 + dt + 1) * 128, :], in_=o
            )


def split_drain_waits(nc):
    """walrus CoreV3 rejects >1 sync wait on several instruction types;
    split extras into single-wait NOPs preceding the instruction on the
    same (in-order) engine."""
    idx = 0

    def fix_block(b):
        nonlocal idx
        new = []
        changed = False
        for inst in b.instructions:
            si = inst.sync_info
            if si is not None and si.on_wait and len(si.on_wait) > 1:
                waits = list(si.on_wait)
                for w in waits[:-1]:
                    idx += 1
                    nop = mybir.InstNoOp(
                        name=f"I-dsplit-{idx}",
                        sync_info=mybir.SyncInfo(on_wait=[w], on_update=[]),
                    )
                    nop.engine = inst.engine
                    new.append(nop)
                inst.sync_info = mybir.SyncInfo(
                    on_wait=[waits[-1]], on_update=list(si.on_update or [])
                )
                changed = True
            new.append(inst)
        if changed:
            b.instructions = new

    for f in nc.m.functions:
        for b in f.blocks:
            fix_block(b)


def declare_io(nc, cfg: Cfg):
    c = cfg
    WD = getattr(mybir.dt, c.mmdt)
    ND = c.D // 128
    NFF = c.DFF // 128
    spec = {
        "x_fm": ([c.D, c.T_kv], WD, False),
        "xq32": ([c.D, c.T_q], F32, False),
        "wq": ([ND, 128, ND, 128], WD, False),
        "wk": ([ND, 128, ND, 128], WD, False),
        "wv": ([c.D, c.D], WD, False),
        "bq": ([c.D], F32, False),
        "bk": ([c.D], F32, False),
        "vb": ([c.D], F32, False),
        "wproj": ([c.D, c.D], WD, False),
        "wfc1": ([NFF, 128, ND, 128], WD, False),
        "bfc1": ([c.DFF], F32, False),
        "wfc2": ([2, NFF, 128, 512], WD, False),
        "maskA": ([128, c.NKTA * 256], WD, False),
        "maskB": ([128, c.NKTB * 256], WD, False),
        "out": ([c.D, c.T_q], F32, True),
        "dbg_k": ([128, c.T_kv], WD, True),
        "dbg_q": ([128, 2 * c.T_q], WD, True),
        "dbg_v": ([128, c.VA], WD, True),
        "dbg_o": ([128, c.T_q], WD, True),
        "dbg_x": ([128, 512], WD, True),
        "dbg_den": ([2 * c.HP, 2, 512], WD, True),
        "dbg_mask": ([128, c.NKTA * 256], WD, True),
        "dbg_pt": ([128, 1536], WD, True),
    }
    io = {}
    for name, (shape, dt, is_out) in spec.items():
        io[name] = nc.declare_dram_parameter(name, shape, dt, isOutput=is_out).ap()
    return io


def build(cfg: Cfg, split: bool = True):
    nc = bass.Bass(num_devices=8)
    io = declare_io(nc, cfg)
    with tile.TileContext(nc) as tc:
        decoder_kernel(tc, cfg, io)
    if split:
        split_drain_waits(nc)
    return nc


# ======================= host-side prep =======================


def make_masks(cfg: Cfg, qg, nkt):
    """[128, nkt*256] fp16: 0 where key k <= query q (valid), else -60000."""
    m = np.zeros((128, nkt * 256), np.float32)
    q = qg + np.arange(cfg.CH)[None, :]
    for k in range(nkt):
        kg = k * 128 + np.arange(128)[:, None]
        m[:, k * 256 : (k + 1) * 256] = (kg > q).astype(np.float32) * MASK_NEG
    return m.astype(np.float16)


def host_prep(cfg: Cfg, x, ln1_g, ln1_b, w_qkv, w_proj, ln2_g, ln2_b, w_fc1, w_fc2):
    """Returns (in_maps list of 8 dicts, assemble(results)->full out)."""
    D, H, DH = cfg.D, cfg.H, cfg.DH
    ND, NFF = D // 128, cfg.DFF // 128
    x = np.asarray(x, np.float32)
    B = x.shape[0]
    w_qkv = np.asarray(w_qkv, np.float32)
    bqkv = np.asarray(ln1_b, np.float32) @ w_qkv
    w_qkv = w_qkv * np.asarray(ln1_g, np.float32)[:, None]
    s = 1.0 / np.sqrt(DH).astype(np.float32)
    bq = bqkv[0:D] * s
    bk = bqkv[D : 2 * D]
    bv = bqkv[2 * D : 3 * D]
    wq = w_qkv[:, 0:D] * s
    wk = w_qkv[:, D : 2 * D]
    wv = w_qkv[:, 2 * D : 3 * D]
    bfc1 = np.asarray(ln2_b, np.float32) @ np.asarray(w_fc1, np.float32)
    wfc1 = np.asarray(w_fc1, np.float32) * np.asarray(ln2_g, np.float32)[:, None]
    wfc2 = np.asarray(w_fc2, np.float32)

    wd = np.float32 if cfg.mmdt == "float32" else np.float16
    pack_kc = lambda w: np.ascontiguousarray(
        w.reshape(ND, 128, -1, 128).transpose(2, 1, 0, 3).astype(wd)
    )  # w[kt*128+p, o*128+c] -> [o, p, kt, c]
    weights = {
        "wq": pack_kc(wq),
        "wk": pack_kc(wk),
        "wv": wv.astype(wd),
        "bq": bq.astype(np.float32),
        "bk": bk.astype(np.float32),
        "vb": bv.astype(np.float32),
        "wproj": np.asarray(w_proj, np.float32).astype(wd),
        "wfc1": pack_kc(wfc1),
        "bfc1": bfc1.astype(np.float32),
        "wfc2": np.ascontiguousarray(
            wfc2.reshape(NFF, 128, 2, 512).transpose(2, 0, 1, 3).astype(wd)
        ),
    }

    in_maps = []
    core_rows = []
    n_j = 4
    for c in range(8):
        b, j = c // n_j, c % n_j
        qgA, qgB = cfg.CH * j, cfg.CH * (2 * n_j - 1 - j)
        rows = np.r_[qgA : qgA + cfg.CH, qgB : qgB + cfg.CH]
        core_rows.append((b, rows))
        im = dict(weights)
        im["x_fm"] = np.ascontiguousarray(x[b].T.astype(wd))
        im["xq32"] = np.ascontiguousarray(x[b][rows].T)
        im["maskA"] = make_masks(cfg, qgA, cfg.NKTA)
        im["maskB"] = make_masks(cfg, qgB, cfg.NKTB)
        in_maps.append(im)

    def assemble(results):
        out = np.zeros((B, x.shape[1], D), np.float32)
        for c, (b, rows) in enumerate(core_rows):
            out[b][rows] = results[c]["out"].T
        return out

    return in_maps, assemble


# ======================= public entry point =======================

LAST_RESULTS = {}
_CACHE = {}


def kernel(x, ln1_g, ln1_b, w_qkv, w_proj, ln2_g, ln2_b, w_fc1, w_fc2,
           _trace=False):
    """Full-input decoder block on 8 TRN2 NeuronCores; returns full output."""
    from concourse.bass_utils import run_bass_kernel_spmd

    cfg = Cfg()
    in_maps, assemble = host_prep(
        cfg, x, ln1_g, ln1_b, w_qkv, w_proj, ln2_g, ln2_b, w_fc1, w_fc2
    )
    if "nc" not in _CACHE:
        _CACHE["nc"] = build(cfg)
    res = run_bass_kernel_spmd(
        _CACHE["nc"], in_maps, core_ids=list(range(8)), trace=_trace
    )
    LAST_RESULTS["res"] = res
    return assemble(res.results)
